# revision 1
# baseline (speedup 1.0000x reference)
"""GCN policy network (2x GCNConv + global max pool + linear head) on 8 TRN2
NeuronCores.

Sharding: nodes split into 8 windows of NW; core c owns window c. Each core
aggregates messages for its own nodes. The (dis-scaled, transformed) node
feature table is AllGathered into a transposed SBUF layout
[128 partitions = 8 windows x 16 features, NW nodes]; per-edge gathers run on
the 8 GPSIMD cores (ap_gather), one Q7 group per source window. Edge slots are
grouped by per-(core, src-window) destination degree so segment sums become
uniform-stride tensor_reduce ops; a small second ap_gather realigns the
bucket-ordered partial sums to canonical node order, and one constant
block-ones matmul sums the 8 window partials. Global max pool is a free-dim
reduce + AllReduce(max); the linear head runs replicated.
"""

import numpy as np

import concourse.bass as bass
import concourse.mybir as mybir
import concourse.bacc as bacc
import concourse.tile as tile
from concourse import bass_utils

F_IN = 128
H = 16
A = 10
N_CORES = 8


def _plan(edge_index: np.ndarray, n_nodes: int, nw: int, chunk: int):
    """Host-side sharding/layout prep: CSR-style bucketed slot plan.

    Returns per-core index tensors plus the (globally uniform) instruction
    plan. Only graph *structure* is computed here (counts / orderings); all
    arithmetic on feature data happens on device.
    """
    src = edge_index[0].astype(np.int64)
    dst = edge_index[1].astype(np.int64)
    ntot = nw * N_CORES

    deg_in = np.bincount(dst, minlength=n_nodes).astype(np.float64)

    # per (core, group): edges with dst in window core, src in window group
    core_of = dst // nw
    grp_of = src // nw
    dstloc = (dst % nw).astype(np.int32)
    srcloc = (src % nw).astype(np.int32)

    # counts per (c, g, dstloc)
    sub = {}
    for c in range(N_CORES):
        mc = core_of == c
        for g in range(N_CORES):
            m = mc & (grp_of == g)
            sub[(c, g)] = (srcloc[m], dstloc[m])

    # per-(c,g) degree histograms
    kmap = {}
    kmax = 1
    for key, (s, d) in sub.items():
        if len(d):
            cnt = np.bincount(d, minlength=nw)
        else:
            cnt = np.zeros(nw, np.int64)
        kmap[key] = cnt
        kmax = max(kmax, int(cnt.max()) if len(d) else 1)

    # global bucket capacities: n_k = max over (c,g) of #dsts with count k
    nk = np.zeros(kmax + 1, np.int64)
    for key, cnt in kmap.items():
        nz = cnt[cnt > 0]
        if len(nz):
            h = np.bincount(nz, minlength=kmax + 1)
            nk = np.maximum(nk, h)

    ks = [k for k in range(1, kmax + 1) if nk[k] > 0]

    # instruction plan shared by every (core, group):
    # gather instrs: list of (slot_off, n_idx, [(rel_off, m, k, col_off), ...])
    instrs = []
    cur = []  # spans in current gather instr
    cur_len = 0
    slot_off = 0
    col_off = 1  # col 0 reserved as zero column
    bucket_info = []  # (k, col_off, n_k, [(span list refs)])
    for k in ks:
        n = int(nk[k])
        bucket_info.append((k, col_off, n))
        done = 0
        while done < n:
            m = min(n - done, max(1, chunk // k))
            span_len = m * k
            if cur_len + span_len > chunk and cur_len > 0:
                pad = (-cur_len) % 32
                instrs.append((slot_off, cur_len + pad, cur))
                slot_off += cur_len + pad
                cur, cur_len = [], 0
            cur.append((cur_len, m, k, col_off + done))
            cur_len += span_len
            done += m
        col_off += n
    if cur_len:
        pad = (-cur_len) % 32
        instrs.append((slot_off, cur_len + pad, cur))
        slot_off += cur_len + pad
    total_slots = slot_off
    ncols = col_off  # includes zero col
    assert ncols < 32767, ncols

    # per-core idx / ridx arrays
    idx_all = np.zeros((N_CORES, N_CORES, total_slots), np.int16)  # [c, g, L]
    ridx_all = np.zeros((N_CORES, N_CORES, nw), np.int16)  # [c, g, nw]
    for c in range(N_CORES):
        for g in range(N_CORES):
            s, d = sub[(c, g)]
            cnt = kmap[(c, g)]
            if len(d) == 0:
                continue
            order = np.lexsort((s, d))
            s, d = s[order], d[order]
            # nodes grouped by their count k, ascending dst within
            dvals = np.unique(d)
            kofd = cnt[dvals]
            # starting position of each dst's run inside the (sorted-by-dst) list
            runstart = np.concatenate([[0], np.cumsum(kofd)[:-1]])
            # for each bucket k: nodes with kofd == k, in ascending dst order
            for (k, coff, n) in bucket_info:
                selnodes = dvals[kofd == k]
                if len(selnodes) == 0:
                    continue
                pos = runstart[kofd == k]
                # column index for node i of this bucket: coff + i
                ridx_all[c, g, selnodes] = (coff + np.arange(len(selnodes))).astype(
                    np.int16
                )
                # slot positions: found from the instr plan spans for bucket k
                bi = 0
                for (soff, nidx, spans) in instrs:
                    for (roff, m, kk, co) in spans:
                        if kk != k:
                            continue
                        lo = co - coff
                        hi = lo + m
                        nodes_here = selnodes[(lo <= np.arange(len(selnodes))) &
                                              (np.arange(len(selnodes)) < hi)]
                        idxs_here = np.arange(len(selnodes))[lo:hi]
                        if len(idxs_here) == 0:
                            continue
                        # slots for these nodes
                        base = soff + roff
                        for j, ni in enumerate(idxs_here):
                            st = pos[ni]
                            idx_all[c, g, base + j * k : base + j * k + k] = s[
                                st : st + k
                            ].astype(np.int16)
                _ = bi

    # wrap into ap_gather layout: idx j of group g -> [16g + j%16, j//16]
    def wrap(arr, c):
        L = arr.shape[-1]
        Lc = (L + 15) // 16 * 16
        out = np.zeros((128, Lc // 16), np.int16)
        for g in range(N_CORES):
            a = np.zeros(Lc, np.int16)
            a[:L] = arr[c, g]
            out[16 * g : 16 * g + 16, :] = a.reshape(Lc // 16, 16).T
        return out

    idx_w = [wrap(idx_all, c) for c in range(N_CORES)]
    # ridx: same wrap (values 0 = zero col)
    ridx_w = [wrap(ridx_all, c) for c in range(N_CORES)]

    deg_full = np.full(ntot, 1e30, np.float32)
    deg_full[:n_nodes] = (deg_in + 1.0).astype(np.float32)

    return instrs, total_slots, ncols, idx_w, ridx_w, deg_full


def _build(nw, total_slots, ncols, instrs, chunk):
    ntot = nw * N_CORES
    nc = bacc.Bacc("TRN2", target_bir_lowering=False, debug=False,
                   num_devices=N_CORES)
    dt = mybir.dt.float32

    xT_in = nc.dram_tensor("xT", [F_IN, nw], dt, kind="ExternalInput")
    deg_in = nc.dram_tensor("degv", [1, nw], dt, kind="ExternalInput")
    ones_in = nc.dram_tensor("ones16", [1, 16], dt, kind="ExternalInput")
    ident_in = nc.dram_tensor("ident", [128, 128], dt, kind="ExternalInput")
    idx_in = nc.dram_tensor("idxs", [128, (total_slots + 15) // 16], mybir.dt.int16,
                            kind="ExternalInput")
    ridx_in = nc.dram_tensor("ridxs", [128, (nw + 15) // 16], mybir.dt.int16,
                             kind="ExternalInput")
    blk_in = nc.dram_tensor("blk", [128, 16], dt, kind="ExternalInput")
    w1_in = nc.dram_tensor("W1", [F_IN, H], dt, kind="ExternalInput")
    b1_in = nc.dram_tensor("b1", [H, 1], dt, kind="ExternalInput")
    w2_in = nc.dram_tensor("W2", [H, H], dt, kind="ExternalInput")
    b2_in = nc.dram_tensor("b2", [H, 1], dt, kind="ExternalInput")
    wc_in = nc.dram_tensor("Wc", [H, A], dt, kind="ExternalInput")
    bc_in = nc.dram_tensor("bc", [1, A], dt, kind="ExternalInput")
    out_t = nc.dram_tensor("out", [1, A], dt, kind="ExternalOutput")

    NCH = min(512, nw)  # column chunk for matmul/elementwise stages
    RCH = 896 if nw % 896 == 0 else nw  # realign chunk

    with tile.TileContext(nc) as tc:
        with (
            tc.tile_pool(name="persist", bufs=1) as pp,
            tc.tile_pool(name="tabp", bufs=1) as tabp,
            tc.tile_pool(name="hp", bufs=1) as hp,
            tc.tile_pool(name="partp", bufs=1) as partp,
            tc.tile_pool(name="msgp", bufs=1) as msgp,
            tc.tile_pool(name="alnp", bufs=2) as alnp,
            tc.tile_pool(name="smallp", bufs=1) as smallp,
            tc.tile_pool(name="psum", bufs=2, space="PSUM") as psp,
            tc.tile_pool(name="dram", bufs=1, space="DRAM") as dram,
        ):
            # --- persistent small tensors
            w1 = pp.tile([F_IN, H], dt)
            nc.sync.dma_start(w1[:], w1_in[:])
            w2 = pp.tile([H, H], dt)
            nc.sync.dma_start(w2[:], w2_in[:])
            wc = pp.tile([H, A], dt)
            nc.sync.dma_start(wc[:], wc_in[:])
            b1 = pp.tile([H, 1], dt)
            nc.sync.dma_start(b1[:], b1_in[:])
            b2 = pp.tile([H, 1], dt)
            nc.sync.dma_start(b2[:], b2_in[:])
            bcb = pp.tile([1, A], dt)
            nc.sync.dma_start(bcb[:], bc_in[:])
            blk = pp.tile([128, 16], dt)
            nc.sync.dma_start(blk[:], blk_in[:])
            ones16 = pp.tile([1, 16], dt)
            nc.sync.dma_start(ones16[:], ones_in[:])
            idxs = pp.tile([128, (total_slots + 15) // 16], mybir.dt.int16)
            nc.sync.dma_start(idxs[:], idx_in[:])
            ridxs = pp.tile([128, (nw + 15) // 16], mybir.dt.int16)
            nc.sync.dma_start(ridxs[:], ridx_in[:])

            dis_dram = dram.tile([1, nw], dt, tag="disd")
            for j in range(0, nw, 512):
                cw = min(512, nw - j)
                degc = smallp.tile([1, 512], dt, tag="degc")
                nc.sync.dma_start(degc[:, :cw], deg_in[:, j:j + cw])
                nc.vector.reciprocal(degc[:, :cw], degc[:, :cw])
                disc = smallp.tile([1, 512], dt, tag="disc")
                nc.scalar.activation(disc[:, :cw], degc[:, :cw],
                                     mybir.ActivationFunctionType.Sqrt)
                nc.sync.dma_start(dis_dram[:, j:j + cw], disc[:, :cw])

            hcur = None  # [16, nw] sbuf tile holding current layer activations

            for layer in range(2):
                w = w1 if layer == 0 else w2
                bvec = b1 if layer == 0 else b2
                kdim = F_IN if layer == 0 else H

                ag_in = dram.tile([16, nw], dt, tag=f"agin{layer}")
                ag_out = dram.tile([128, nw], dt, tag=f"agout{layer}")

                # local scaled table S_T = dis * (W.T @ rhs), [16, cols] blocks
                sb = nc.enter_named_scope(f"sbuild{layer}", False)
                for j in range(0, nw, 512):
                    cw = min(512, nw - j)
                    if layer == 0:
                        xc = smallp.tile([F_IN, 512], dt, tag="xc")
                        nc.sync.dma_start(xc[:, :cw], xT_in[:, j:j + cw])
                        rhs_ap = xc[:kdim, :cw]
                    else:
                        rhs_ap = hcur[:kdim, j:j + cw]
                    ps = psp.tile([16, 512], dt, tag="mma")
                    nc.tensor.matmul(ps[:, :cw], lhsT=w[:kdim, :], rhs=rhs_ap,
                                     start=True, stop=True)
                    disc = smallp.tile([1, 512], dt, tag="disc")
                    nc.sync.dma_start(disc[:, :cw], dis_dram[:, j:j + cw])
                    psd = psp.tile([16, 512], dt, tag="mmd")
                    nc.tensor.matmul(psd[:, :cw], lhsT=ones16[:],
                                     rhs=disc[:, :cw], start=True, stop=True)
                    ts = smallp.tile([16, 512], dt, tag="sc")
                    nc.vector.tensor_copy(ts[:, :cw], ps[:, :cw])
                    sc = smallp.tile([16, 512], dt, tag="sct")
                    nc.vector.tensor_mul(sc[:, :cw], ts[:, :cw], psd[:, :cw])
                    nc.sync.dma_start(ag_in[:, j:j + cw], sc[:, :cw])

                nc.leave_named_scope(f"sbuild{layer}", sb[0], False)
                agc = nc.enter_named_scope(f"ag{layer}", False)
                nc.gpsimd.collective_compute(
                    "AllGather", mybir.AluOpType.bypass,
                    replica_groups=[list(range(N_CORES))],
                    ins=[ag_in.opt()], outs=[ag_out.opt()],
                )
                nc.leave_named_scope(f"ag{layer}", agc[0], False)

                table = tabp.tile([128, nw], dt, tag="table")
                nc.sync.dma_start(table[:], ag_out[:])

                partial = partp.tile([128, ncols], dt, tag="partial")
                nc.vector.memset(partial[:, 0:1], 0.0)

                gsc = nc.enter_named_scope(f"gather{layer}", False)
                for (soff, nidx, spans) in instrs:
                    msg = msgp.tile([128, chunk], dt, tag="msg")
                    nc.gpsimd.ap_gather(
                        out_ap=msg[:, :nidx],
                        in_ap=table[:],
                        idxs_ap=idxs[:, soff // 16:(soff + nidx) // 16],
                        channels=128, num_elems=nw, d=1, num_idxs=nidx,
                    )
                    for (roff, m, k, coff) in spans:
                        if k == 1:
                            nc.vector.tensor_copy(
                                partial[:, coff:coff + m],
                                msg[:, roff:roff + m])
                        else:
                            mv = msg[:, roff:roff + m * k].rearrange(
                                "p (n k) -> p k n", k=k)
                            nc.vector.tensor_add(
                                partial[:, coff:coff + m],
                                mv[:, 0, :], mv[:, 1, :])
                            for jj in range(2, k):
                                nc.vector.tensor_add(
                                    partial[:, coff:coff + m],
                                    partial[:, coff:coff + m],
                                    mv[:, jj, :])

                nc.leave_named_scope(f"gather{layer}", gsc[0], False)
                rsc = nc.enter_named_scope(f"realign{layer}", False)
                hnew = hp.tile([16, nw], dt, tag="h")
                for j0 in range(0, nw, RCH):
                    aln = alnp.tile([128, RCH], dt, tag="aln")
                    nc.gpsimd.ap_gather(
                        out_ap=aln[:],
                        in_ap=partial[:],
                        idxs_ap=ridxs[:, j0 // 16:(j0 + RCH) // 16],
                        channels=128, num_elems=ncols, d=1, num_idxs=RCH,
                    )
                    sob = alnp.tile([16, RCH], dt, tag="sob")
                    nc.sync.dma_start(sob[:], ag_in[:, j0:j0 + RCH])
                    for j1 in range(0, RCH, 512):
                        j = j0 + j1
                        cw = min(512, RCH - j1)
                        ps = psp.tile([16, 512], dt, tag="mma")
                        nc.tensor.matmul(ps[:, :cw], lhsT=blk[:],
                                         rhs=aln[:, j1:j1 + cw],
                                         start=True, stop=True)
                        disc = smallp.tile([1, 512], dt, tag="disc")
                        nc.sync.dma_start(disc[:, :cw], dis_dram[:, j:j + cw])
                        psd = psp.tile([16, 512], dt, tag="mmd")
                        nc.tensor.matmul(psd[:, :cw], lhsT=ones16[:],
                                         rhs=disc[:, :cw],
                                         start=True, stop=True)
                        u = smallp.tile([16, 512], dt, tag="acc")
                        nc.vector.tensor_add(u[:, :cw], sob[:, j1:j1 + cw],
                                             ps[:, :cw])
                        v = smallp.tile([16, 512], dt, tag="tso")
                        nc.vector.tensor_mul(v[:, :cw], u[:, :cw], psd[:, :cw])
                        nc.scalar.activation(hnew[:, j:j + cw], v[:, :cw],
                                             mybir.ActivationFunctionType.Relu,
                                             bias=bvec[:])
                nc.leave_named_scope(f"realign{layer}", rsc[0], False)
                hcur = hnew

            # global max pool over own nodes, then across cores
            pooled = smallp.tile([16, 1], dt, tag="pool")
            nc.vector.tensor_reduce(pooled[:], hcur[:],
                                    axis=mybir.AxisListType.X,
                                    op=mybir.AluOpType.max)
            pin = dram.tile([16, 1], dt, tag="pin")
            pout = dram.tile([16, 1], dt, tag="pout")
            nc.sync.dma_start(pin[:], pooled[:])
            nc.gpsimd.collective_compute(
                "AllReduce", mybir.AluOpType.max,
                replica_groups=[list(range(N_CORES))],
                ins=[pin.opt()], outs=[pout.opt()],
            )
            pooled2 = smallp.tile([16, 1], dt, tag="pool2")
            nc.sync.dma_start(pooled2[:], pout[:])
            ps = psp.tile([1, A], dt, tag="mmc")
            nc.tensor.matmul(ps[:], lhsT=pooled2[:], rhs=wc[:],
                             start=True, stop=True)
            ores = smallp.tile([1, A], dt, tag="ores")
            nc.vector.tensor_add(ores[:], ps[:], bcb[:])
            nc.sync.dma_start(out_t[:], ores[:])

    nc.compile()
    return nc


def kernel(x, edge_index, W1, b1, W2, b2, Wc, bc, _nw=12544, _chunk=2048,
           _run=None):
    n_nodes = x.shape[0]
    nw = _nw
    instrs, total_slots, ncols, idx_w, ridx_w, deg_full = _plan(
        np.asarray(edge_index), n_nodes, nw, _chunk)

    nc = _build(nw, total_slots, ncols, instrs, _chunk)

    x = np.asarray(x, np.float32)
    blk = np.zeros((128, 16), np.float32)
    blk[np.arange(128), np.arange(128) % 16] = 1.0

    in_maps = []
    for c in range(N_CORES):
        xw = np.zeros((nw, F_IN), np.float32)
        lo, hi = c * nw, min((c + 1) * nw, n_nodes)
        if hi > lo:
            xw[: hi - lo] = x[lo:hi]
        in_maps.append({
            "xT": np.ascontiguousarray(xw.T),
            "degv": deg_full[c * nw:(c + 1) * nw][None, :].copy(),
            "ones16": np.ones((1, 16), np.float32),
            "idxs": idx_w[c],
            "ridxs": ridx_w[c],
            "blk": blk,
            "ident": np.eye(128, dtype=np.float32),
            "W1": np.asarray(W1, np.float32),
            "b1": np.asarray(b1, np.float32).reshape(H, 1),
            "W2": np.asarray(W2, np.float32),
            "b2": np.asarray(b2, np.float32).reshape(H, 1),
            "Wc": np.asarray(Wc, np.float32),
            "bc": np.asarray(bc, np.float32).reshape(1, A),
        })

    if _run == "sim":
        from concourse.bass_interp import MultiCoreSim
        sim = MultiCoreSim(nc, num_cores=N_CORES, trace=False)
        for c in range(N_CORES):
            for k, v in in_maps[c].items():
                sim.cores[c].tensor(k)[:] = v
        sim.simulate()
        return np.asarray(sim.cores[0].tensor("out")).reshape(A)

    res = bass_utils.run_bass_kernel_spmd(
        nc, in_maps, core_ids=list(range(N_CORES)),
        trace=bool(_run == "trace"))
    if _run == "trace":
        print("HW exec time:", res.exec_time_ns, "ns")
    return np.asarray(res.results[0]["out"]).reshape(A)

